# revision 1
# baseline (speedup 1.0000x reference)
"""Trainium2 Bass kernel for CrossAttention (B=8, N=M=2048, C=1024), fp32.

Sharding: data-parallel — one batch element per NeuronCore (8 cores).

Per-core computation (batch b):
  q  = x[b] @ wq^T          -> held transposed:  qT[d, n]
  kT[d, m] = (support[b] @ wk^T)^T
  v[m, d]  = (support[b] @ wv^T) * mask[m]   (post-softmax column mask == row
                                              mask on v; softmax denominator
                                              still spans all m)
  p  = exp(SCALE * qT^T kT)  (no max subtraction: logits ~ N(0, 8), safe fp32)
  o[n, d] = (p @ v) / rowsum(p)
  The reference's  out.swapaxes(1,2).reshape(N, C)  permutation satisfies
  o_perm[2t+i, c] = o[1024*i + c, t], so the final projection becomes
  final[2t+i, d'] = sum_c o[1024*i + c, t] * proj_w[d', c]  — a plain matmul
  with o-block-i rows as the contraction dim, written out with row stride 2.

Matmul operands are float32r (full-rate PE streaming; plain fp32 is 4 cyc/row).
Host-side prep transposes x/support/weights once (fp32 has no DMA-transpose on
TRN2) and lays weights out in consumption order so chunked DMAs pipeline with
the first accumulation groups at phase boundaries.
"""

import sys

sys.path.insert(0, "/opt/trn_rl_repo")

import numpy as np

import concourse.bass as bass
import concourse.tile as tile
from concourse import bacc, mybir
from concourse.bass_utils import run_bass_kernel_spmd
from concourse.masks import make_identity

F32 = mybir.dt.float32
F32R = mybir.dt.float32r
AF = mybir.ActivationFunctionType

B, N, M, C = 8, 2048, 2048, 1024
CT = C // 128          # 8 c-tiles (contraction / channel partition tiles)
MT = M // 128          # 16 m-tiles
SCALE = (C // 8) ** -0.5
NCHUNK = 256           # q rows computed per chunk in the attention phase
MS = 512               # m-chunk for kv build and the s matmul free dim
WCH = 16               # weight DMA chunks (arrival order == consumption order)

_CACHE = {}


def _build_program():
    nc = bacc.Bacc(
        "TRN2",
        target_bir_lowering=False,
        debug=False,
        enable_asserts=False,
        num_devices=8,
    )

    xT = nc.dram_tensor("xT", [128, CT, N], F32, kind="ExternalInput")
    sT = nc.dram_tensor("sT", [128, CT, M], F32, kind="ExternalInput")
    wqT = nc.dram_tensor("wqT", [128, CT * C], F32, kind="ExternalInput")
    wkT = nc.dram_tensor("wkT", [128, CT * C], F32, kind="ExternalInput")
    wvT = nc.dram_tensor("wvT", [128, CT * C], F32, kind="ExternalInput")
    pwT = nc.dram_tensor("pwT", [128, CT * C], F32, kind="ExternalInput")
    maskf = nc.dram_tensor("maskf", [128, MT], F32, kind="ExternalInput")
    biasb = nc.dram_tensor("biasb", [128, C], F32, kind="ExternalInput")
    out = nc.dram_tensor("out", [N, C], F32, kind="ExternalOutput")

    with tile.TileContext(nc, pool_alloc_mode="queue") as tc:
        _trace_kernel(tc, xT, sT, wqT, wkT, wvT, pwT, maskf, biasb, out)
    nc.compile()
    return nc


def _dma_w(nc, wtile, wdram, ch0=0, ch1=WCH):
    # chunked weight load: pipelines with the first consuming matmul groups
    cw = (CT * C) // WCH
    for ch in range(ch0, ch1):
        nc.sync.dma_start(
            wtile[:, ch * cw:(ch + 1) * cw],
            wdram[:, ch * cw:(ch + 1) * cw].bitcast(F32R),
        )


def _dma_act(nc, atile, adram, col0, cols):
    # per-ct chunked activation load (arrival order == psum-group order)
    for ct in range(CT):
        nc.sync.dma_start(
            atile[:, ct, :],
            adram[:, ct, col0:col0 + cols].bitcast(F32R),
        )


def _trace_kernel(tc, xT, sT, wqT, wkT, wvT, pwT, maskf, biasb, out):
    nc = tc.nc

    from contextlib import ExitStack

    with ExitStack() as ctx:
        persist = ctx.enter_context(tc.tile_pool(name="persist", bufs=1))

        ident0 = persist.tile([128, 128], F32, tag="ident0")
        make_identity(nc, ident0[:])
        ident = persist.tile([128, 128], F32R, tag="ident")
        nc.scalar.copy(ident[:], ident0[:])
        maskt = persist.tile([128, MT], F32, tag="maskt")
        nc.sync.dma_start(maskt[:], maskf[:])

        # o bounce buffer in DRAM (dependency-tracked tile)
        dram = ctx.enter_context(tc.tile_pool(name="dram", bufs=1, space="DRAM"))
        o_dram = dram.tile([N, C], F32, tag="o_dram")

        # v/kT live through attention; released before proj.  kT's pool opens
        # at phase K so phase V has room to preload wk alongside wv.
        kv_ctx = ctx.enter_context(ExitStack())
        vp = kv_ctx.enter_context(tc.tile_pool(name="vp", bufs=1))
        # v [m,d] as 16 col-blocks of [128, C]
        v = vp.tile([128, MT * C], F32R, tag="v")
        wk_ctx = ctx.enter_context(ExitStack())
        wkp = wk_ctx.enter_context(tc.tile_pool(name="wkp", bufs=1, side="right"))
        wk = wkp.tile([128, CT * C], F32R, tag="wk")

        # ---------------- phase V: v[m, d] = mask * (support @ wv^T) -------
        # wv is dc-major: [p, dc*4096 + ct*512 + dd]
        with (
            tc.tile_pool(name="wvp", bufs=1) as wvp,
            tc.tile_pool(name="stv", bufs=2) as stp,
            tc.tile_pool(name="vps", bufs=3, space="PSUM") as vps,
        ):
            wv = wvp.tile([128, CT * C], F32R, tag="wv")
            st0 = stp.tile([128, CT, MS], F32R, tag="st")
            cw = (CT * C) // WCH
            for i in range(CT):
                nc.sync.dma_start(
                    wv[:, i * cw:(i + 1) * cw],
                    wvT[:, i * cw:(i + 1) * cw].bitcast(F32R),
                )
                nc.sync.dma_start(
                    st0[:, i, :], sT[:, i, 0:MS].bitcast(F32R)
                )
            _dma_w(nc, wv, wvT, CT, WCH)
            for mc in range(M // MS):
                if mc == 0:
                    st = st0
                else:
                    st = stp.tile([128, CT, MS], F32R, tag="st")
                    _dma_act(nc, st, sT, mc * MS, MS)
                # spread next phase's weight prefetch across V's mc loop
                _dma_w(nc, wk, wkT, mc * 4, (mc + 1) * 4)
                for dc in range(C // 512):
                    for j in range(MS // 128):
                        mt = mc * (MS // 128) + j
                        ps = vps.tile([128, 512], F32, tag="vps")
                        for ct in range(CT):
                            nc.tensor.matmul(
                                ps[:],
                                lhsT=st[:, ct, j * 128:(j + 1) * 128],
                                rhs=wv[:, dc * 4096 + ct * 512: dc * 4096 + (ct + 1) * 512],
                                start=(ct == 0),
                                stop=(ct == CT - 1),
                            )
                        nc.vector.tensor_scalar_mul(
                            v[:, mt * C + dc * 512: mt * C + (dc + 1) * 512],
                            ps[:],
                            maskt[:, mt:mt + 1],
                        )

        # ---------------- phase K: kT[d, m] = (support @ wk^T)^T -----------
        # wk is dt-major: [p, dt*1024 + ct*128 + dd]; preloaded during V
        kTp = kv_ctx.enter_context(tc.tile_pool(name="kTp", bufs=1))
        # kT [d,m] as 8 col-blocks of [128, M]
        kT = kTp.tile([128, CT * M], F32R, tag="kT")
        with (
            tc.tile_pool(name="stk", bufs=2) as stp,
            tc.tile_pool(name="kps", bufs=3, space="PSUM") as kps,
        ):
            for mc in range(M // MS):
                st = stp.tile([128, CT, MS], F32R, tag="st")
                _dma_act(nc, st, sT, mc * MS, MS)
                for dt in range(CT):
                    ps = kps.tile([128, MS], F32, tag="kps")
                    for ct in range(CT):
                        nc.tensor.matmul(
                            ps[:],
                            lhsT=wk[:, dt * C + ct * 128: dt * C + (ct + 1) * 128],
                            rhs=st[:, ct, :],
                            start=(ct == 0),
                            stop=(ct == CT - 1),
                        )
                    nc.scalar.copy(
                        kT[:, dt * M + mc * MS: dt * M + (mc + 1) * MS], ps[:]
                    )

        wk_ctx.close()

        # ---------------- attention: per n-chunk qT, then s/p/o ------------
        # wq is dt-major like wk
        with (
            tc.tile_pool(name="wqp", bufs=1) as wqp,
            tc.tile_pool(name="xq", bufs=1) as xqp,
            tc.tile_pool(name="qt", bufs=1) as qtp,
            tc.tile_pool(name="qps", bufs=2, space="PSUM") as qps,
            tc.tile_pool(name="sps", bufs=2, space="PSUM") as sps,
            tc.tile_pool(name="ptps", bufs=2, space="PSUM") as ptps,
            tc.tile_pool(name="ops", bufs=1, space="PSUM") as ops,
            tc.tile_pool(name="psb", bufs=2) as psbp,
            tc.tile_pool(name="ptsb", bufs=2) as ptsbp,
            tc.tile_pool(name="osb", bufs=2) as osbp,
            tc.tile_pool(name="stat", bufs=4) as statp,
        ):
            wq = wqp.tile([128, CT * C], F32R, tag="wq")
            _dma_w(nc, wq, wqT, 0, 2)  # dt0 block: first qT group's weights
            xq0 = xqp.tile([128, CT, NCHUNK], F32R, tag="xq")
            _dma_act(nc, xq0, xT, 0, NCHUNK)
            _dma_w(nc, wq, wqT, 2, WCH)
            for nch in range(N // NCHUNK):
                if nch == 0:
                    xq = xq0
                else:
                    xq = xqp.tile([128, CT, NCHUNK], F32R, tag="xq")
                    _dma_act(nc, xq, xT, nch * NCHUNK, NCHUNK)
                qt = qtp.tile([128, CT * NCHUNK], F32R, tag="qt")
                for dt in range(CT):
                    ps = qps.tile([128, NCHUNK], F32, tag="qps")
                    for ct in range(CT):
                        nc.tensor.matmul(
                            ps[:],
                            lhsT=wq[:, dt * C + ct * 128: dt * C + (ct + 1) * 128],
                            rhs=xq[:, ct, :],
                            start=(ct == 0),
                            stop=(ct == CT - 1),
                        )
                    nc.scalar.copy(
                        qt[:, dt * NCHUNK:(dt + 1) * NCHUNK], ps[:]
                    )
                for nt2 in range(NCHUNK // 128):
                    ntile = nch * (NCHUNK // 128) + nt2
                    partials = statp.tile([128, 4], F32, tag="partials")
                    o_ps = ops.tile([128, C], F32, tag="ops")
                    for g in range(M // MS):
                        s_ps = sps.tile([128, MS], F32, tag="sps")
                        for dt in range(CT):
                            nc.tensor.matmul(
                                s_ps[:],
                                lhsT=qt[:, dt * NCHUNK + nt2 * 128: dt * NCHUNK + (nt2 + 1) * 128],
                                rhs=kT[:, dt * M + g * MS: dt * M + (g + 1) * MS],
                                start=(dt == 0),
                                stop=(dt == CT - 1),
                            )
                        p_sb = psbp.tile([128, MS], F32R, tag="psb")
                        nc.scalar.activation(
                            p_sb[:], s_ps[:], AF.Exp,
                            scale=float(SCALE),
                            accum_out=partials[:, g:g + 1],
                        )
                        pt_ps = ptps.tile([128, MS], F32R, tag="ptps")
                        for j in range(MS // 128):
                            nc.tensor.transpose(
                                pt_ps[:, j * 128:(j + 1) * 128],
                                p_sb[:, j * 128:(j + 1) * 128],
                                ident[:],
                            )
                        pt_sb = ptsbp.tile([128, MS], F32R, tag="ptsb")
                        nc.vector.tensor_copy(pt_sb[:], pt_ps[:])
                        for j in range(MS // 128):
                            mt = g * (MS // 128) + j
                            for dc in range(C // 512):
                                nc.tensor.matmul(
                                    o_ps[:, dc * 512:(dc + 1) * 512],
                                    lhsT=pt_sb[:, j * 128:(j + 1) * 128],
                                    rhs=v[:, mt * C + dc * 512: mt * C + (dc + 1) * 512],
                                    start=(mt == 0),
                                    stop=(mt == MT - 1),
                                )
                    denom = statp.tile([128, 1], F32, tag="denom")
                    nc.vector.reduce_sum(
                        denom[:], partials[:], axis=mybir.AxisListType.X
                    )
                    recip = statp.tile([128, 1], F32, tag="recip")
                    nc.vector.reciprocal(recip[:], denom[:])
                    o_sb = osbp.tile([128, C], F32, tag="osb")
                    nc.vector.tensor_scalar_mul(o_sb[:], o_ps[:], recip[:])
                    nc.sync.dma_start(
                        o_dram[ntile * 128:(ntile + 1) * 128, :], o_sb[:]
                    )

        kv_ctx.close()

        # ---------------- projection with the swapaxes/reshape fold --------
        # pw is dc-major like wv
        with (
            tc.tile_pool(name="pwp", bufs=1) as pwp,
            tc.tile_pool(name="bp", bufs=1) as bp,
            tc.tile_pool(name="obp", bufs=2) as obp,
            tc.tile_pool(name="fps", bufs=2, space="PSUM") as fps,
            tc.tile_pool(name="fsb", bufs=2) as fsbp,
        ):
            pw = pwp.tile([128, CT * C], F32R, tag="pw")
            bias = bp.tile([128, C], F32, tag="bias")
            ob0 = obp.tile([128, CT * C], F32R, tag="ob")
            cw = (CT * C) // WCH
            for i in range(CT):
                nc.sync.dma_start(
                    pw[:, i * cw:(i + 1) * cw],
                    pwT[:, i * cw:(i + 1) * cw].bitcast(F32R),
                )
                # plain 2D slices: a rearranged AP on a DRAM pool tile defeats
                # Tile's RAW dep tracking (read would race the o_dram writes)
                nc.sync.dma_start(
                    ob0[:, i * C:(i + 1) * C],
                    o_dram[i * 128:(i + 1) * 128, :].bitcast(F32R),
                )
            _dma_w(nc, pw, pwT, CT, WCH)
            nc.sync.dma_start(bias[:], biasb[:])
            out_v = out[:].rearrange("(t two) d -> two t d", two=2)
            for i in range(2):
                if i == 0:
                    ob = ob0
                else:
                    ob = obp.tile([128, CT * C], F32R, tag="ob")
                    for ct in range(CT):
                        nc.sync.dma_start(
                            ob[:, ct * C:(ct + 1) * C],
                            o_dram[i * C + ct * 128: i * C + (ct + 1) * 128, :].bitcast(F32R),
                        )
                for dc in range(C // 512):
                    for tt in range(CT):
                        ps = fps.tile([128, 512], F32, tag="fps")
                        for ct in range(CT):
                            nc.tensor.matmul(
                                ps[:],
                                lhsT=ob[:, ct * C + tt * 128: ct * C + (tt + 1) * 128],
                                rhs=pw[:, dc * 4096 + ct * 512: dc * 4096 + (ct + 1) * 512],
                                start=(ct == 0),
                                stop=(ct == CT - 1),
                            )
                        f_sb = fsbp.tile([128, 512], F32, tag="fsb")
                        nc.vector.tensor_add(
                            f_sb[:], ps[:], bias[:, dc * 512:(dc + 1) * 512]
                        )
                        nc.sync.dma_start(
                            out_v[i, tt * 128:(tt + 1) * 128, dc * 512:(dc + 1) * 512],
                            f_sb[:],
                        )


def _prep_w_lhs(w):
    # lhsT weights (wk, wq): dt-major [p, dt*1024 + ct*128 + dd]
    wt = w.T.reshape(CT, 128, CT, 128)          # [ct, p, dt, dd]
    return np.ascontiguousarray(
        wt.transpose(1, 2, 0, 3).reshape(128, CT * C)
    )


def _prep_w_rhs(w):
    # rhs weights (wv, pw): dc-major [p, dc*4096 + ct*512 + dd]
    wt = w.T.reshape(CT, 128, C // 512, 512)    # [ct, p, dc, dd]
    return np.ascontiguousarray(
        wt.transpose(1, 2, 0, 3).reshape(128, CT * C)
    )


def _prep_act(a):
    # a [rows, C] -> a.T [C, rows] grouped as [p, ct, rows]
    n = a.shape[0]
    return np.ascontiguousarray(a.T.reshape(CT, 128, n).transpose(1, 0, 2))


def prep_in_maps(x, support, attn_mask, qkv_w, proj_w, proj_b):
    x = np.asarray(x, dtype=np.float32)
    support = np.asarray(support, dtype=np.float32)
    attn_mask = np.asarray(attn_mask)
    qkv_w = np.asarray(qkv_w, dtype=np.float32)
    proj_w = np.asarray(proj_w, dtype=np.float32)
    proj_b = np.asarray(proj_b, dtype=np.float32)

    wq = _prep_w_lhs(qkv_w[:C])
    wk = _prep_w_lhs(qkv_w[C:2 * C])
    wv = _prep_w_rhs(qkv_w[2 * C:])
    pw = _prep_w_rhs(proj_w)
    maskf = np.ascontiguousarray(
        attn_mask.astype(np.float32).reshape(MT, 128).T
    )
    biasb = np.ascontiguousarray(np.broadcast_to(proj_b, (128, C)))

    in_maps = []
    for b in range(B):
        in_maps.append({
            "xT": _prep_act(x[b]),
            "sT": _prep_act(support[b]),
            "wqT": wq,
            "wkT": wk,
            "wvT": wv,
            "pwT": pw,
            "maskf": maskf,
            "biasb": biasb,
        })
    return in_maps


def kernel(x, support, attn_mask, qkv_w, proj_w, proj_b):
    if "nc" not in _CACHE:
        _CACHE["nc"] = _build_program()
    nc = _CACHE["nc"]

    in_maps = prep_in_maps(x, support, attn_mask, qkv_w, proj_w, proj_b)
    res = run_bass_kernel_spmd(nc, in_maps, core_ids=list(range(B)))
    return np.stack([res.results[b]["out"] for b in range(B)], axis=0)



# revision 10
# speedup vs baseline: 1.3298x; 1.3298x over previous
"""Trainium2 Bass kernel for CrossAttention (B=8, N=M=2048, C=1024), fp32 in/out.

Sharding: data-parallel — one batch element per NeuronCore (8 cores).

Key optimizations over the straightforward version:
  * Mask packing: the reference applies a binary mask over support positions
    AFTER softmax, so masked positions only matter for the softmax denominator.
    The host permutes support rows so unmasked rows come first (1012 -> padded
    to 1024 = mt_u tiles); v is built and p@v contracted over that half only,
    while the s = q@k^T logits (and the exp-sum denominator) still span all M.
    This halves the v-build, the p transposes, and the p@v contraction.
  * bf16 operand storage everywhere (fp32 PSUM accumulation): same PE rate as
    f32r on TRN2 (1 cyc/row) but half the DMA bytes and SBUF footprint, which
    lets qT, kT, v, and the full o tensor stay SBUF-resident.  o in its
    natural [n, d] layout is exactly the lhsT the output projection needs
    (the swapaxes/reshape fold makes proj contract over o's row index), so
    no DRAM bounce is needed.
  * Q phase first: its first matmul needs only a small wq slice + one x
    chunk, so the tensor engine starts ~3us earlier, and the entire V/K
    working set (support^T, wv, wk) prefetches during Q's compute.
  * Few large DMA instructions (multi-dim APs) — each dma_start costs
    ~650ns of serial issue on the sync sequencer.
  * The attention inner loop is software-pipelined: transposes + p@v of
    group g-1 are emitted after the s matmuls of group g, hiding the exp
    latency from the tensor engine.

Per-core computation (batch b):
  qT[d, n] = (x[b] @ wq^T)^T
  v[m, d]  = (support_perm[b] @ wv^T) * mask_perm[m]   (m < mv only)
  kT[d, m] = (support_perm[b] @ wk^T)^T                (all m)
  p = exp(SCALE * q k^T)  (no max subtraction: logits ~ N(0, 8), safe fp32)
  o[n, d] = (p[:, :mv] @ v) / rowsum_all_m(p)
  out[2t+i, d'] = sum_c o[1024 i + c, t] * proj_w[d', c] + proj_b[d']
"""

import sys

sys.path.insert(0, "/opt/trn_rl_repo")

import numpy as np
import ml_dtypes

import concourse.bass as bass
import concourse.tile as tile
from concourse import bacc, mybir
from concourse.bass_utils import run_bass_kernel_spmd
from concourse.masks import make_identity

F32 = mybir.dt.float32
BF16 = mybir.dt.bfloat16
AF = mybir.ActivationFunctionType
NPBF = ml_dtypes.bfloat16

B, N, M, C = 8, 2048, 2048, 1024
CT = C // 128          # 8 c-tiles (contraction / channel partition tiles)
MT = M // 128          # 16 m-tiles
SCALE = (C // 8) ** -0.5
NCHUNK = 256           # q rows computed per chunk in the q phase
MS = 512               # m-chunk for kv build and the s matmul free dim

_CACHE = {}


def _build_program(mt_u):
    nc = bacc.Bacc(
        "TRN2",
        target_bir_lowering=False,
        debug=False,
        enable_asserts=False,
        num_devices=8,
    )

    xT = nc.dram_tensor("xT", [128, CT, N], BF16, kind="ExternalInput")
    sT = nc.dram_tensor("sT", [128, CT, M], BF16, kind="ExternalInput")
    wqT = nc.dram_tensor("wqT", [128, CT * C], BF16, kind="ExternalInput")
    wkT = nc.dram_tensor("wkT", [128, CT * C], BF16, kind="ExternalInput")
    wvT = nc.dram_tensor("wvT", [128, CT * C], BF16, kind="ExternalInput")
    pwT = nc.dram_tensor("pwT", [128, CT * C], BF16, kind="ExternalInput")
    maskf = nc.dram_tensor("maskf", [128, mt_u], F32, kind="ExternalInput")
    biasb = nc.dram_tensor("biasb", [128, C], F32, kind="ExternalInput")
    out = nc.dram_tensor("out", [N, C], F32, kind="ExternalOutput")

    with tile.TileContext(nc, pool_alloc_mode="queue") as tc:
        _trace_kernel(tc, mt_u, xT, sT, wqT, wkT, wvT, pwT, maskf, biasb, out)
    nc.compile()
    return nc


def _trace_kernel(tc, mt_u, xT, sT, wqT, wkT, wvT, pwT, maskf, biasb, out):
    nc = tc.nc
    mv = mt_u * 128

    from contextlib import ExitStack

    with ExitStack() as ctx:
        persist = ctx.enter_context(tc.tile_pool(name="persist", bufs=1))

        ident0 = persist.tile([128, 128], F32, tag="ident0")
        make_identity(nc, ident0[:])
        ident = persist.tile([128, 128], BF16, tag="ident")
        nc.scalar.copy(ident[:], ident0[:])

        # o[n, d] persists through to proj: 16 row-tiles of [128, C] bf16
        o_sb = persist.tile([128, (N // 128) * C], BF16, tag="o_sb")

        # qT/v/kT live through attention; pw until proj ends
        qt_ctx = ctx.enter_context(ExitStack())
        qtp = qt_ctx.enter_context(tc.tile_pool(name="qtp", bufs=1))
        qt = qtp.tile([128, CT, N], BF16, tag="qt")
        kv_ctx = ctx.enter_context(ExitStack())
        vp = kv_ctx.enter_context(tc.tile_pool(name="vp", bufs=1))
        v = vp.tile([128, mt_u, C], BF16, tag="v")
        kTp = kv_ctx.enter_context(tc.tile_pool(name="kTp", bufs=1))
        kT = kTp.tile([128, CT, M], BF16, tag="kT")
        pwp = ctx.enter_context(tc.tile_pool(name="pwp", bufs=1, side="right"))
        pw = pwp.tile([128, CT * C], BF16, tag="pw")

        # support^T loaded once, shared by the V and K phases
        st_ctx = ctx.enter_context(ExitStack())
        stp = st_ctx.enter_context(tc.tile_pool(name="stp", bufs=1))
        st = stp.tile([128, CT, M], BF16, tag="st")

        maskt = persist.tile([128, mt_u], F32, tag="maskt")
        bias = persist.tile([128, C], F32, tag="bias")

        # ---------------- phase Q: qT[d, n] = (x @ wq^T)^T -----------------
        # wq is dt-major: [p, dt*1024 + ct*128 + dd]
        with (
            tc.tile_pool(name="wqp", bufs=1, side="right") as wqp,
            tc.tile_pool(name="xq", bufs=2) as xqp,
            tc.tile_pool(name="qps", bufs=3, space="PSUM") as qps,
        ):
            wq = wqp.tile([128, CT * C], BF16, tag="wq")

            # ramp-up: small first slice of wq, then x chunks 0/1, then the
            # rest of wq, then support^T for the K/V phases while Q computes.
            nc.sync.dma_start(wq[:, 0:1024], wqT[:, 0:1024])
            xq0 = xqp.tile([128, CT, NCHUNK], BF16, tag="xq")
            nc.sync.dma_start(xq0[:, :, :], xT[:, :, 0:NCHUNK])
            xq1 = xqp.tile([128, CT, NCHUNK], BF16, tag="xq")
            nc.sync.dma_start(xq1[:, :, :], xT[:, :, NCHUNK:2 * NCHUNK])
            nc.sync.dma_start(wq[:, 1024:4096], wqT[:, 1024:4096])
            nc.sync.dma_start(wq[:, 4096:8192], wqT[:, 4096:8192])
            nc.sync.dma_start(maskt[:], maskf[:])
            for mc in range(M // MS):
                nc.sync.dma_start(
                    st[:, :, mc * MS:(mc + 1) * MS],
                    sT[:, :, mc * MS:(mc + 1) * MS],
                )
            nc.sync.dma_start(bias[:], biasb[:])

            for nch in range(N // NCHUNK):
                if nch == 0:
                    xq = xq0
                elif nch == 1:
                    xq = xq1
                else:
                    xq = xqp.tile([128, CT, NCHUNK], BF16, tag="xq")
                    nc.sync.dma_start(
                        xq[:, :, :], xT[:, :, nch * NCHUNK:(nch + 1) * NCHUNK]
                    )
                for dt in range(CT):
                    ps = qps.tile([128, NCHUNK], F32, tag="qps")
                    for ct in range(CT):
                        nc.tensor.matmul(
                            ps[:],
                            lhsT=wq[:, dt * C + ct * 128: dt * C + (ct + 1) * 128],
                            rhs=xq[:, ct, :],
                            start=(ct == 0),
                            stop=(ct == CT - 1),
                        )
                    nc.scalar.copy(
                        qt[:, dt, nch * NCHUNK:(nch + 1) * NCHUNK], ps[:]
                    )

        # ---------------- phases K then V (shared support^T) ---------------
        with (
            tc.tile_pool(name="wkp", bufs=1, side="right") as wkp,
            tc.tile_pool(name="wvp", bufs=1, side="right") as wvp,
            tc.tile_pool(name="kvps", bufs=3, space="PSUM") as kvps,
        ):
            wk = wkp.tile([128, CT * C], BF16, tag="wk")
            wv = wvp.tile([128, CT * C], BF16, tag="wv")
            # wk first (K consumes dt-blocks in order), wv + pw behind it
            nc.sync.dma_start(wk[:, 0:2048], wkT[:, 0:2048])
            nc.sync.dma_start(wk[:, 2048:4096], wkT[:, 2048:4096])
            nc.sync.dma_start(wk[:, 4096:8192], wkT[:, 4096:8192])
            nc.sync.dma_start(wv[:, 0:4096], wvT[:, 0:4096])
            nc.sync.dma_start(wv[:, 4096:8192], wvT[:, 4096:8192])
            nc.sync.dma_start(pw[:, 0:4096], pwT[:, 0:4096])
            nc.sync.dma_start(pw[:, 4096:8192], pwT[:, 4096:8192])

            # K: kT[d, m] = (support @ wk^T)^T; wk is dt-major like wq
            for mc in range(M // MS):
                for dt in range(CT):
                    ps = kvps.tile([128, MS], F32, tag="kvps")
                    for ct in range(CT):
                        nc.tensor.matmul(
                            ps[:],
                            lhsT=wk[:, dt * C + ct * 128: dt * C + (ct + 1) * 128],
                            rhs=st[:, ct, mc * MS:(mc + 1) * MS],
                            start=(ct == 0),
                            stop=(ct == CT - 1),
                        )
                    nc.scalar.copy(
                        kT[:, dt, mc * MS:(mc + 1) * MS], ps[:]
                    )

            # V: v[m, d] = mask * (support @ wv^T); wv is dc-major; first mv
            # rows only
            for mt in range(mt_u):
                for dc in range(C // 512):
                    ps = kvps.tile([128, 512], F32, tag="kvps")
                    for ct in range(CT):
                        nc.tensor.matmul(
                            ps[:],
                            lhsT=st[:, ct, mt * 128:(mt + 1) * 128],
                            rhs=wv[:, dc * 4096 + ct * 512: dc * 4096 + (ct + 1) * 512],
                            start=(ct == 0),
                            stop=(ct == CT - 1),
                        )
                    nc.vector.tensor_scalar_mul(
                        v[:, mt, dc * 512:(dc + 1) * 512],
                        ps[:],
                        maskt[:, mt:mt + 1],
                    )

        st_ctx.close()

        # ---------------- attention: s / exp / transpose / p@v -------------
        with (
            tc.tile_pool(name="sps", bufs=2, space="PSUM") as sps,
            tc.tile_pool(name="ptps", bufs=2, space="PSUM") as ptps,
            tc.tile_pool(name="ops", bufs=1, space="PSUM") as ops,
            tc.tile_pool(name="psb", bufs=3) as psbp,
            tc.tile_pool(name="ptsb", bufs=2) as ptsbp,
            tc.tile_pool(name="stat", bufs=4) as statp,
        ):
            def transpose_and_pv(g, p_sb, o_ps):
                pt_ps = ptps.tile([128, MS], BF16, tag="ptps")
                for j in range(MS // 128):
                    nc.tensor.transpose(
                        pt_ps[:, j * 128:(j + 1) * 128],
                        p_sb[:, j * 128:(j + 1) * 128],
                        ident[:],
                    )
                pt_sb = ptsbp.tile([128, MS], BF16, tag="ptsb")
                nc.vector.tensor_copy(pt_sb[:], pt_ps[:])
                for j in range(MS // 128):
                    mt = g * (MS // 128) + j
                    for dc in range(C // 512):
                        nc.tensor.matmul(
                            o_ps[:, dc * 512:(dc + 1) * 512],
                            lhsT=pt_sb[:, j * 128:(j + 1) * 128],
                            rhs=v[:, mt, dc * 512:(dc + 1) * 512],
                            start=(mt == 0),
                            stop=(mt == mt_u - 1),
                        )

            for ntile in range(N // 128):
                partials = statp.tile([128, 4], F32, tag="partials")
                o_ps = ops.tile([128, C], F32, tag="ops")
                pending = None  # software pipeline: hide exp latency
                for g in range(M // MS):
                    s_ps = sps.tile([128, MS], F32, tag="sps")
                    for dt in range(CT):
                        nc.tensor.matmul(
                            s_ps[:],
                            lhsT=qt[:, dt, ntile * 128:(ntile + 1) * 128],
                            rhs=kT[:, dt, g * MS:(g + 1) * MS],
                            start=(dt == 0),
                            stop=(dt == CT - 1),
                        )
                    p_sb = psbp.tile([128, MS], BF16, tag="psb")
                    nc.scalar.activation(
                        p_sb[:], s_ps[:], AF.Exp,
                        scale=float(SCALE),
                        accum_out=partials[:, g:g + 1],
                    )
                    if pending is not None:
                        transpose_and_pv(*pending, o_ps)
                        pending = None
                    if g * MS < mv:
                        pending = (g, p_sb)
                if pending is not None:
                    transpose_and_pv(*pending, o_ps)
                denom = statp.tile([128, 1], F32, tag="denom")
                nc.vector.reduce_sum(
                    denom[:], partials[:], axis=mybir.AxisListType.X
                )
                recip = statp.tile([128, 1], F32, tag="recip")
                nc.vector.reciprocal(recip[:], denom[:])
                nc.vector.tensor_scalar_mul(
                    o_sb[:, ntile * C:(ntile + 1) * C], o_ps[:], recip[:]
                )

        kv_ctx.close()
        qt_ctx.close()

        # ---------------- projection with the swapaxes/reshape fold --------
        # out[2t+i, d'] = sum_c o[1024 i + c, t] pw[d', c] + bias: the lhsT
        # blocks are o's SBUF row-tiles as produced by attention.  pw is
        # dc-major like wv.
        with (
            tc.tile_pool(name="fps", bufs=4, space="PSUM") as fps,
            tc.tile_pool(name="fsb", bufs=2) as fsbp,
        ):
            out_v = out[:].rearrange("(t two) d -> two t d", two=2)
            for i in range(2):
                for dc in range(C // 512):
                    for tt in range(CT):
                        ps = fps.tile([128, 512], F32, tag="fps")
                        for ct in range(CT):
                            nc.tensor.matmul(
                                ps[:],
                                lhsT=o_sb[:, (CT * i + ct) * C + tt * 128: (CT * i + ct) * C + (tt + 1) * 128],
                                rhs=pw[:, dc * 4096 + ct * 512: dc * 4096 + (ct + 1) * 512],
                                start=(ct == 0),
                                stop=(ct == CT - 1),
                            )
                        f_sb = fsbp.tile([128, 512], F32, tag="fsb")
                        nc.vector.tensor_add(
                            f_sb[:], ps[:], bias[:, dc * 512:(dc + 1) * 512]
                        )
                        nc.sync.dma_start(
                            out_v[i, tt * 128:(tt + 1) * 128, dc * 512:(dc + 1) * 512],
                            f_sb[:],
                        )


def _prep_w_lhs(w):
    # lhsT weights (wk, wq): dt-major [p, dt*1024 + ct*128 + dd]
    wt = w.T.reshape(CT, 128, CT, 128)          # [ct, p, dt, dd]
    return np.ascontiguousarray(
        wt.transpose(1, 2, 0, 3).reshape(128, CT * C).astype(NPBF)
    )


def _prep_w_rhs(w):
    # rhs weights (wv, pw): dc-major [p, dc*4096 + ct*512 + dd]
    wt = w.T.reshape(CT, 128, C // 512, 512)    # [ct, p, dc, dd]
    return np.ascontiguousarray(
        wt.transpose(1, 2, 0, 3).reshape(128, CT * C).astype(NPBF)
    )


def _prep_act(a):
    # a [rows, C] -> a.T [C, rows] grouped as [p, ct, rows]
    n = a.shape[0]
    return np.ascontiguousarray(
        a.T.reshape(CT, 128, n).transpose(1, 0, 2).astype(NPBF)
    )


def _mask_perm(attn_mask):
    # permutation packing unmasked support rows first; tile count for packed v
    mask = np.asarray(attn_mask)
    perm = np.argsort(mask == 0, kind="stable")
    cnt = int((mask != 0).sum())
    mt_u = max(1, min(MT, -(-cnt // 128)))
    return perm, mt_u


def prep_in_maps(x, support, attn_mask, qkv_w, proj_w, proj_b):
    x = np.asarray(x, dtype=np.float32)
    support = np.asarray(support, dtype=np.float32)
    attn_mask = np.asarray(attn_mask)
    qkv_w = np.asarray(qkv_w, dtype=np.float32)
    proj_w = np.asarray(proj_w, dtype=np.float32)
    proj_b = np.asarray(proj_b, dtype=np.float32)

    perm, mt_u = _mask_perm(attn_mask)
    maskp = attn_mask[perm].astype(np.float32)

    wq = _prep_w_lhs(qkv_w[:C])
    wk = _prep_w_lhs(qkv_w[C:2 * C])
    wv = _prep_w_rhs(qkv_w[2 * C:])
    pw = _prep_w_rhs(proj_w)
    maskf = np.ascontiguousarray(
        maskp[:mt_u * 128].reshape(mt_u, 128).T
    )
    biasb = np.ascontiguousarray(np.broadcast_to(proj_b, (128, C)))

    in_maps = []
    for b in range(B):
        in_maps.append({
            "xT": _prep_act(x[b]),
            "sT": _prep_act(support[b][perm]),
            "wqT": wq,
            "wkT": wk,
            "wvT": wv,
            "pwT": pw,
            "maskf": maskf,
            "biasb": biasb,
        })
    return in_maps


def kernel(x, support, attn_mask, qkv_w, proj_w, proj_b):
    _, mt_u = _mask_perm(attn_mask)
    if ("nc", mt_u) not in _CACHE:
        _CACHE[("nc", mt_u)] = _build_program(mt_u)
        _CACHE["nc"] = _CACHE[("nc", mt_u)]
    nc = _CACHE[("nc", mt_u)]

    in_maps = prep_in_maps(x, support, attn_mask, qkv_w, proj_w, proj_b)
    res = run_bass_kernel_spmd(nc, in_maps, core_ids=list(range(B)))
    return np.stack([res.results[b]["out"] for b in range(B)], axis=0)
